# revision 1
# baseline (speedup 1.0000x reference)
"""Trainium2 Bass kernel for GATRelationNet (self-contained).

Math:
  att_h = attributes @ att_w                        [N, H]
  e     = leaky_relu(att_h@a1 + (att_h@a2).T, 0.2)  [N, N]
  attn  = softmax(e, axis=1)
  att_outs = attn @ att_h                           [N, H]
  img_proj = image_feats @ img_w                    [B, H]
  sem_proj = att_outs @ sem_w + sem_b               [N, H]
  out[b,n] = fc_b + sum_h fc_w[h]*relu(img_proj[b,h] + sem_proj[n,h])

Strategy (8 cores):
  - Replicate the GAT on every core (transposed layouts, unnormalized
    softmax: colsum via PE ones-matmul, normalization folded into the
    sem2 PSUM->SBUF copy).
  - Shard the relation part over the batch dim (32 rows/core). The
    [B,N,H] hidden tensor is never materialized in DRAM: relu tiles
    [128h, 1000n] are produced in SBUF by ScalarE/VectorE/GPSIMD and
    immediately reduced over h by PE matmuls with masked fc_w columns
    as the stationary operand (row b of the PSUM out tile accumulates
    batch b; other rows add exact zeros).
  - Large GAT matmuls run in float32r (1 PE cycle/col vs 4 for fp32,
    ~1e-4 precision); operands are rounded on device by ACT/DVE-copy
    producers as the BIR verifier requires. The relation reduce runs
    in fp16 (DVE/GPSIMD cannot round to f32r; fp16 keeps 10 mantissa
    bits at the same 1 cycle/col).
"""

import numpy as np
import ml_dtypes

import concourse.bass as bass
import concourse.mybir as mybir
import concourse.tile as tile
from concourse import bacc
from concourse.bass_utils import run_bass_kernel_spmd

P = 128
B, N, A, H, IDIM = 256, 1000, 512, 512, 512
NCORES = 8
BS = B // NCORES      # 32 batch rows per core
KA = A // P           # 4 contraction chunks over A
HM = H // P           # 4 h chunks
NJ = 8                # j (class, softmax-reduced) chunks
JW = N // NJ          # 125
IW = 500              # i half width (PSUM bank = 512 fp32)
NEG = 0.2

# relation relu n-split between engines: [0,SA)=ScalarE, [SA,SA+SD)=VectorE,
# rest = GPSIMD. SD even (keeps DVE packed write modes).
SA = 160
SD = 624
SG = N - SA - SD

F32 = mybir.dt.float32
F32R = mybir.dt.float32r
F16 = mybir.dt.float16
AF = mybir.ActivationFunctionType
OP = mybir.AluOpType

_CACHE = {}


def _build_program():
    if "nc" in _CACHE:
        return _CACHE["nc"]

    nc = bacc.Bacc(
        "TRN2", target_bir_lowering=False, debug=False, num_devices=NCORES
    )

    d_attrT = nc.dram_tensor("attrT", [A, N], F32, kind="ExternalInput")
    d_att_w = nc.dram_tensor("att_w", [P, KA * H], F32, kind="ExternalInput")
    d_w12 = nc.dram_tensor("w12", [P, 2 * KA], F32, kind="ExternalInput")
    d_img_w = nc.dram_tensor("img_w", [P, KA * H], F32, kind="ExternalInput")
    d_imgfT = nc.dram_tensor("imgfT", [P, KA * BS], F32, kind="ExternalInput")
    d_sem_w = nc.dram_tensor("sem_w", [P, KA * H], F32, kind="ExternalInput")
    d_sem_bT = nc.dram_tensor("sem_bT", [P, HM], F32, kind="ExternalInput")
    # masked fc_w (fp16): for (m, b), [128, BS] tile, col b = fc_w chunk
    d_fcwm2 = nc.dram_tensor(
        "fcwm", [HM * P, BS * BS], F16, kind="ExternalInput"
    )
    d_fc_b = nc.dram_tensor("fc_b", [1, 1], F32, kind="ExternalInput")
    d_out = nc.dram_tensor("out", [BS, N], F32, kind="ExternalOutput")

    with tile.TileContext(nc) as tc:
        _program(
            nc, tc, d_attrT, d_att_w, d_w12, d_img_w, d_imgfT, d_sem_w,
            d_sem_bT, d_fcwm2, d_fc_b, d_out,
        )

    nc.compile()
    _CACHE["nc"] = nc
    return nc


def _program(nc, tc, d_attrT, d_att_w, d_w12, d_img_w, d_imgfT, d_sem_w,
             d_sem_bT, d_fcwm2, d_fc_b, d_out):
    cpool_ctx = tc.tile_pool(name="consts", bufs=1)
    cpool = cpool_ctx.__enter__()
    epool_ctx = tc.tile_pool(name="etmp", bufs=2)
    epool = epool_ctx.__enter__()
    # staging pool: DMA-landing + GAT-input tensors, released after phase A
    lpool_ctx = tc.tile_pool(name="loadp", bufs=1)
    lpool = lpool_ctx.__enter__()
    rawpool_ctx = tc.tile_pool(name="raw", bufs=4)
    rawpool = rawpool_ctx.__enter__()

    # ---- load inputs; round matmul operands to f32r via DVE copies ----
    attrT = [lpool.tile([P, N], F32R, tag=f"attrT{k}", name=f"attrT{k}")
             for k in range(KA)]
    attwa = lpool.tile([P, KA * H], F32R, tag="attwa", name="attwa")
    att_w = [attwa[:, k * H:(k + 1) * H] for k in range(KA)]
    w12a_raw = lpool.tile([P, 2 * KA], F32, tag="w12raw", name="w12raw")
    w12a = lpool.tile([P, 2 * KA], F32R, tag="w12a", name="w12a")
    w12 = [w12a[:, 2 * k:2 * (k + 1)] for k in range(KA)]
    semwa = cpool.tile([P, KA * H], F32R, tag="semwa", name="semwa")
    sem_w = [semwa[:, k * H:(k + 1) * H] for k in range(KA)]
    imgwa = cpool.tile([P, KA * H], F32, tag="imgwa", name="imgwa")
    img_w = [imgwa[:, k * H:(k + 1) * H] for k in range(KA)]
    imgfTa = cpool.tile([P, KA * BS], F32, tag="imgfTa", name="imgfTa")
    imgfT = [imgfTa[:, k * BS:(k + 1) * BS] for k in range(KA)]
    sem_bTa = cpool.tile([P, HM], F32, tag="sembTa", name="sembTa")
    sem_bT = [sem_bTa[:, m:m + 1] for m in range(HM)]
    fwm = [cpool.tile([P, BS * BS], F16, tag=f"fwm{m}", name=f"fwm{m}")
           for m in range(HM)]
    fcb = cpool.tile([1, 1], F32, tag="fcb", name="fcb")

    def load_round(dsrc, dst, sl, width):
        raw = rawpool.tile([P, N], F32, tag="raw", name="raw")
        nc.sync.dma_start(raw[:, 0:width], dsrc[sl, :])
        nc.vector.tensor_copy(dst[:], raw[:, 0:width])

    nc.sync.dma_start(w12a_raw[:], d_w12[:, :])
    nc.vector.tensor_copy(w12a[:], w12a_raw[:])
    for k in range(KA):
        sl = slice(k * P, (k + 1) * P)
        load_round(d_attrT, attrT[k], sl, N)
    nc.sync.dma_start(fcb[:], d_fc_b[:, :])

    ones_row = cpool.tile([1, P], F32, tag="ones_row", name="ones_row")
    nc.vector.memset(ones_row[:], 1.0)
    ones_row_r = cpool.tile([1, P], F32R, tag="ones_row_r", name="ones_row_r")
    nc.vector.tensor_copy(ones_row_r[:], ones_row[:])
    ones_col = cpool.tile([P, 1], F32, tag="ones_col", name="ones_col")
    nc.vector.memset(ones_col[:], 1.0)
    ones_col_r = cpool.tile([P, 1], F32R, tag="ones_col_r", name="ones_col_r")
    nc.vector.tensor_copy(ones_col_r[:], ones_col[:])

    # persistent GAT tensors
    att_h = [cpool.tile([JW, H], F32R, tag=f"atth{j}", name=f"atth{j}")
             for j in range(NJ)]
    expT = [cpool.tile([JW, N], F32R, tag=f"expT{j}", name=f"expT{j}")
            for j in range(NJ)]
    f1row = cpool.tile([1, N], F32R, tag="f1row", name="f1row")
    f1b = cpool.tile([P, N], F32, tag="f1b", name="f1b")
    f2col = [cpool.tile([JW, 1], F32, tag=f"f2col{j}", name=f"f2col{j}")
             for j in range(NJ)]
    imgb = [cpool.tile([P, BS], F32, tag=f"imgb{m}", name=f"imgb{m}")
            for m in range(HM)]
    aoT = [cpool.tile([P, N], F32R, tag=f"aoT{m}", name=f"aoT{m}")
           for m in range(HM)]
    rb_sb = [cpool.tile([P, IW], F32, tag=f"rb{ih}", name=f"rb{ih}")
             for ih in range(2)]
    sem2T = [cpool.tile([P, N], F32, tag=f"sem2T{m}", name=f"sem2T{m}")
             for m in range(HM)]
    fcb_rep = cpool.tile([BS, 1], F32, tag="fcb_rep", name="fcb_rep")
    out_sb = cpool.tile([BS, N], F32, tag="out_sb", name="out_sb")

    # warm up the gpsimd tensor_scalar ucode op early (op load is ~us)
    gps_warm = cpool.tile([P, 8], F32, tag="gpswarm", name="gpswarm")
    nc.vector.memset(gps_warm[:], 0.0)
    nc.gpsimd.tensor_scalar(
        gps_warm[:], gps_warm[:], 0.0, 0.0, op0=OP.add, op1=OP.max
    )

    # ---- phase A: small matmuls (att_h, f1, f2, img_proj, fcb bcast) ----
    with tc.tile_pool(name="psumA", bufs=1, space="PSUM") as psumA:
        # f1 row [1, N] then broadcast to 128 partitions
        for ih in range(2):
            isl = slice(ih * IW, (ih + 1) * IW)
            ps = psumA.tile([1, IW], F32, tag="f1", name="f1")
            for k in range(KA):
                nc.tensor.matmul(
                    ps[:], w12a[:, 2 * k:2 * k + 1], attrT[k][:, isl],
                    start=(k == 0), stop=(k == KA - 1),
                )
            nc.vector.tensor_copy(f1row[:, isl], ps[:])
        for ih in range(2):
            isl = slice(ih * IW, (ih + 1) * IW)
            ps = psumA.tile([P, IW], F32, tag="f1b", name="f1b")
            nc.tensor.matmul(ps[:], ones_row_r[:], f1row[:, isl])
            nc.vector.tensor_copy(f1b[:, isl], ps[:])

        # f2 column per j chunk: Nf=2 (fp32r needs even free counts);
        # col 0 is a byproduct (f1 for these j), col 1 is f2
        for j in range(NJ):
            ps = psumA.tile([JW, 2], F32, tag="f2", name="f2", bufs=2)
            jsl = slice(j * JW, (j + 1) * JW)
            for k in range(KA):
                nc.tensor.matmul(
                    ps[:], attrT[k][:, jsl], w12a[:, 2 * k:2 * k + 2],
                    start=(k == 0), stop=(k == KA - 1),
                )
            nc.vector.tensor_copy(f2col[j][:], ps[:, 1:2])


    raww = rawpool.tile([P, KA * H], F32, tag="raww", name="raww", bufs=1)
    nc.sync.dma_start(raww[:], d_att_w[:, :])
    nc.vector.tensor_copy(attwa[:], raww[:])

    # ---- phase B: e^T -> leaky -> exp, per j chunk (all on ScalarE:
    # Prelu == leaky_relu lives in the same ACT table set as Exp) ----
    for j in range(NJ):
        e_t = epool.tile([JW, N], F32, tag="e", name="e")
        if j % 2 == 0:
            nc.scalar.activation(
                e_t[:], f1b[0:JW, :], AF.Prelu, bias=f2col[j][:, 0:1],
                alpha=NEG,
            )
        else:
            # DVE path: e = f1 + f2, then leaky = max(e, 0.2e)
            nc.vector.tensor_scalar(
                e_t[:], f1b[0:JW, :], f2col[j][:, 0:1], None, op0=OP.add
            )
            nc.vector.scalar_tensor_tensor(
                e_t[:], e_t[:], NEG, e_t[:], op0=OP.mult, op1=OP.max
            )
        nc.scalar.activation(expT[j][:], e_t[:], AF.Exp)

    with tc.tile_pool(name="psumA2", bufs=1, space="PSUM") as psumA2:
        # att_h natural [j, h] (lhsT for the att_outs matmul)
        for j in range(NJ):
            ps = psumA2.tile([JW, H], F32, tag="ah", name="ah", bufs=4)
            jsl = slice(j * JW, (j + 1) * JW)
            for k in range(KA):
                nc.tensor.matmul(
                    ps[:], attrT[k][:, jsl], att_w[k][:],
                    start=(k == 0), stop=(k == KA - 1),
                )
            nc.vector.tensor_copy(att_h[j][:], ps[:])

    # late loads: not needed until phases D/E
    raww2 = rawpool.tile([P, KA * H], F32, tag="raww", name="raww2", bufs=1)
    nc.sync.dma_start(raww2[:], d_sem_w[:, :])
    nc.vector.tensor_copy(semwa[:], raww2[:])
    nc.sync.dma_start(imgwa[:], d_img_w[:, :])
    nc.sync.dma_start(imgfTa[:], d_imgfT[:, :])
    nc.sync.dma_start(sem_bTa[:], d_sem_bT[:, :])
    for m in range(HM):
        sl = slice(m * P, (m + 1) * P)
        nc.sync.dma_start(
            fwm[m][:],
            d_fcwm2[m * P:(m + 1) * P, :],
        )
    nc.sync.dma_start(fcb[:], d_fc_b[:, :])
    rawpool_ctx.__exit__(None, None, None)
    lpool_ctx.__exit__(None, None, None)


    # ---- phase C: att_outs^T (unnormalized) + colsum ----
    # Emission order matters: the bulk ao matmuls go early in the PE queue
    # so they consume expT chunks as phase B produces them; the recip/rb
    # chain (blocked on a DRAM round-trip) is emitted afterwards.
    with tc.tile_pool(name="psumB", bufs=1, space="PSUM") as psumB:
        cs_row = epool.tile([1, N], F32, tag="cs_row", name="cs_row")
        ps_cs = [
            psumB.tile([1, IW], F32, tag=f"cs{ih}", name=f"cs{ih}")
            for ih in range(2)
        ]
        for j in range(NJ):
            for ih in range(2):
                isl = slice(ih * IW, (ih + 1) * IW)
                nc.tensor.matmul(
                    ps_cs[ih][:], ones_col_r[0:JW, :], expT[j][:, isl],
                    start=(j == 0), stop=(j == NJ - 1),
                )
        for ih in range(2):
            nc.vector.tensor_copy(
                cs_row[:, ih * IW:(ih + 1) * IW], ps_cs[ih][:]
            )
        # approximate reciprocal (~2 ULP, ~2.8x faster than the exact
        # iterative divide) directly on the [1, N] row
        recip_f = epool.tile([1, N], F32, tag="recip_f", name="recip_f")
        rc_scr = epool.tile([1, N], F32, tag="rc_scr", name="rc_scr")
        nc.vector.reciprocal_approx_accurate(
            out=recip_f[:], in_=cs_row[:], scratch=rc_scr[:]
        )
        recip_rr = epool.tile([1, N], F32R, tag="recip_rr", name="recip_rr")
        nc.vector.tensor_copy(recip_rr[:], recip_f[:])
        for ih in range(2):
            isl = slice(ih * IW, (ih + 1) * IW)
            for m in range(HM):
                msl = slice(m * P, (m + 1) * P)
                ps_ao = psumB.tile([P, IW], F32, tag="ao", name="ao", bufs=3)
                for j in range(NJ):
                    nc.tensor.matmul(
                        ps_ao[:], att_h[j][:, msl], expT[j][:, isl],
                        start=(j == 0), stop=(j == NJ - 1),
                    )
                nc.scalar.copy(aoT[m][:, isl], ps_ao[:])
        for ih in range(2):
            isl = slice(ih * IW, (ih + 1) * IW)
            ps_rb = psumB.tile([P, IW], F32, tag="rbp", name="rbp", bufs=2)
            nc.tensor.matmul(ps_rb[:], ones_row_r[:], recip_rr[:, isl])
            nc.vector.tensor_copy(rb_sb[ih][:], ps_rb[:])

    # ---- phase A2: img_proj + fcb (emitted after B so the ACT queue
    # isn't head-blocked on the late img_w/imgfT loads) ----
    with tc.tile_pool(name="psumI", bufs=1, space="PSUM") as psumI:
        # img_proj^T [h, b] + sem_b fold (bias for the relation relu)
        for m in range(HM):
            ps = psumI.tile([P, BS], F32, tag="img", name="img", bufs=4)
            msl = slice(m * P, (m + 1) * P)
            for k in range(KA):
                nc.tensor.matmul(
                    ps[:], img_w[k][:, msl], imgfTa[:, k * BS:(k + 1) * BS],
                    start=(k == 0), stop=(k == KA - 1),
                )
            nc.scalar.activation(
                imgb[m][:], ps[:], AF.Identity, bias=sem_bTa[:, m:m + 1]
            )

        # fc_b broadcast to [BS, 1]
        ps = psumI.tile([BS, 1], F32, tag="fcbp", name="fcbp")
        nc.tensor.matmul(ps[:], ones_row[0:1, 0:BS], fcb[0:1, 0:1])
        nc.vector.tensor_copy(fcb_rep[:], ps[:])


    # ---- phase D: sem2^T = (sem_w^T @ ao_unnorm^T) * (1/colsum) ----
    with tc.tile_pool(name="psumC", bufs=2, space="PSUM") as psumC:
        for m in range(HM):
            msl = slice(m * P, (m + 1) * P)
            for ih in range(2):
                isl = slice(ih * IW, (ih + 1) * IW)
                ps = psumC.tile([P, IW], F32, tag="s2", name="s2", bufs=4)
                for k in range(KA):
                    nc.tensor.matmul(
                        ps[:], sem_w[k][:, msl], aoT[k][:, isl],
                        start=(k == 0), stop=(k == KA - 1),
                    )
                nc.vector.tensor_tensor(
                    sem2T[m][:, isl], ps[:], rb_sb[ih][:], op=OP.mult
                )

    epool_ctx.__exit__(None, None, None)
    rpool_ctx = tc.tile_pool(name="relu", bufs=8)
    rpool = rpool_ctx.__enter__()

    # ---- phase E: relation net ----
    with tc.tile_pool(name="psumD", bufs=1, space="PSUM") as psumD:
        out_ps = [
            psumD.tile([BS, IW], F32, tag=f"out{ih}", name=f"out{ih}")
            for ih in range(2)
        ]
        for m in range(HM):
            for b in range(BS):
                r = rpool.tile([P, N], F16, tag="r", name="r")
                bias = imgb[m][:, b:b + 1]
                nc.scalar.activation(
                    r[:, 0:SA], sem2T[m][:, 0:SA], AF.Relu, bias=bias
                )
                nc.vector.tensor_scalar(
                    r[:, SA:SA + SD], sem2T[m][:, SA:SA + SD], bias, 0.0,
                    op0=OP.add, op1=OP.max,
                )
                nc.gpsimd.tensor_scalar(
                    r[:, SA + SD:N], sem2T[m][:, SA + SD:N], bias, 0.0,
                    op0=OP.add, op1=OP.max,
                )
                for ih in range(2):
                    isl = slice(ih * IW, (ih + 1) * IW)
                    nc.tensor.matmul(
                        out_ps[ih][:],
                        fwm[m][:, b * BS:(b + 1) * BS], r[:, isl],
                        start=(m == 0 and b == 0),
                        stop=(m == HM - 1 and b == BS - 1),
                    )
        for ih in range(2):
            isl = slice(ih * IW, (ih + 1) * IW)
            nc.scalar.activation(
                out_sb[:, isl], out_ps[ih][:], AF.Identity,
                bias=fcb_rep[:, 0:1],
            )
    nc.sync.dma_start(d_out[:, :], out_sb[:])

    rpool_ctx.__exit__(None, None, None)
    cpool_ctx.__exit__(None, None, None)


def _prepare_in_maps(image_feats, attributes, att_w, att_a, img_w, sem_w,
                     sem_b, fc_w, fc_b):
    f = np.float32
    attributes = np.asarray(attributes, f)
    att_w = np.asarray(att_w, f)
    att_a = np.asarray(att_a, f)
    image_feats = np.asarray(image_feats, f)

    attrT = np.ascontiguousarray(attributes.T)                     # [A, N]
    a1, a2 = att_a[:H, 0], att_a[H:, 0]
    w12 = np.stack([att_w @ a1, att_w @ a2], axis=1).astype(f)     # [A, 2]
    # pack per-chunk small tensors into single contiguous DMAs:
    # w12 [A,2] -> [128, (k,2)]; sem_b [H] -> [128, (m)]
    w12 = np.ascontiguousarray(
        w12.reshape(KA, P, 2).transpose(1, 0, 2).reshape(P, 2 * KA)
    )
    sem_bT = np.ascontiguousarray(
        np.asarray(sem_b, f).reshape(HM, P).T
    )
    fc_w = np.asarray(fc_w, f).reshape(H)
    fc_b = np.asarray(fc_b, f).reshape(1, 1)
    def pack_k(w):
        return np.ascontiguousarray(
            np.asarray(w, f).reshape(KA, P, H).transpose(1, 0, 2)
            .reshape(P, KA * H)
        )
    img_w = pack_k(img_w)
    sem_w = pack_k(sem_w)
    att_w_packed = pack_k(att_w)
    # masked stationary fc_w tiles: fcwm[m, b, h, b'] = fc_w[m*P+h]*(b'==b)
    fcwm = np.zeros((HM, BS, P, BS), f)
    for m in range(HM):
        for b in range(BS):
            fcwm[m, b, :, b] = fc_w[m * P:(m + 1) * P]
    fcwm = np.ascontiguousarray(
        fcwm.transpose(0, 2, 1, 3).reshape(HM * P, BS * BS).astype(np.float16)
    )

    shared = {
        "attrT": attrT, "att_w": att_w_packed, "w12": w12,
        "img_w": img_w, "sem_w": sem_w, "sem_bT": sem_bT,
        "fcwm": fcwm, "fc_b": fc_b,
    }
    in_maps = []
    for c in range(NCORES):
        # [I, BS] -> [128, (k, BS)] packed
        imgfT = np.ascontiguousarray(
            image_feats[c * BS:(c + 1) * BS, :].T
            .reshape(KA, P, BS).transpose(1, 0, 2).reshape(P, KA * BS)
        )
        in_maps.append(dict(shared, imgfT=imgfT))
    return in_maps


def _make_runner(nc, in_maps):
    """Build the sharded PJRT callable once (mirrors
    bass2jax.run_bass_via_pjrt's multi-core path) so repeated kernel()
    calls reuse the compiled NEFF executable."""
    import jax
    from jax.sharding import Mesh, PartitionSpec

    try:
        from jax.experimental.shard_map import shard_map
    except ImportError:
        shard_map = jax.shard_map
    from concourse import bass2jax

    bass2jax.install_neuronx_cc_hook()
    n_cores = len(in_maps)
    partition_name = (
        nc.partition_id_tensor.name if nc.partition_id_tensor else None
    )
    in_names, out_names, out_avals = [], [], []
    for alloc in nc.m.functions[0].allocations:
        if not isinstance(alloc, mybir.MemoryLocationSet):
            continue
        name = alloc.memorylocations[0].name
        if alloc.kind == "ExternalInput":
            if name != partition_name:
                in_names.append(name)
        elif alloc.kind == "ExternalOutput":
            out_names.append(name)
            out_avals.append(
                jax.core.ShapedArray(
                    tuple(alloc.tensor_shape), mybir.dt.np(alloc.dtype)
                )
            )
    all_in_names = list(in_names) + list(out_names)
    if partition_name is not None:
        all_in_names.append(partition_name)
    n_params, n_outs = len(in_names), len(out_avals)

    def _body(*args):
        operands = list(args)
        if partition_name is not None:
            operands.append(bass2jax.partition_id_tensor())
        return tuple(bass2jax._bass_exec_p.bind(
            *operands,
            out_avals=tuple(out_avals),
            in_names=tuple(all_in_names),
            out_names=tuple(out_names),
            lowering_input_output_aliases=(),
            sim_require_finite=True,
            sim_require_nnan=True,
            nc=nc,
        ))

    donate = tuple(range(n_params, n_params + n_outs))
    devices = jax.devices()[:n_cores]
    mesh = Mesh(np.asarray(devices), ("core",))
    sharded = jax.jit(
        shard_map(
            _body, mesh=mesh,
            in_specs=(PartitionSpec("core"),) * (n_params + n_outs),
            out_specs=(PartitionSpec("core"),) * n_outs,
            check_rep=False,
        ),
        donate_argnums=donate, keep_unused=True,
    )

    import zlib

    def call(maps):
        concat_in = [
            np.concatenate([np.asarray(maps[c][n]) for c in range(n_cores)], 0)
            for n in in_names
        ]
        # keep inputs device-resident across calls with identical data
        key = tuple(zlib.adler32(x.tobytes()) for x in concat_in)
        dev = _CACHE.get("dev_inputs")
        if dev is None or dev[0] != key:
            dev = (key, [jax.device_put(x) for x in concat_in])
            _CACHE["dev_inputs"] = dev
        zeros = [
            np.zeros((n_cores * av.shape[0], *av.shape[1:]), av.dtype)
            for av in out_avals
        ]
        outs = sharded(*dev[1], *zeros)
        jax.block_until_ready(outs)
        oi = out_names.index("out")
        full = np.asarray(outs[oi]).reshape(n_cores, *out_avals[oi].shape)
        return np.concatenate(list(full), axis=0).astype(np.float32)

    return call


def run(inputs, **spmd_kwargs):
    """Returns (full output [B, N], BassKernelResults) via the generic
    run_bass_kernel_spmd path (used by test tooling)."""
    nc = _build_program()
    in_maps = _prepare_in_maps(**inputs)
    res = run_bass_kernel_spmd(nc, in_maps, list(range(NCORES)), **spmd_kwargs)
    out = np.concatenate(
        [res.results[c]["out"] for c in range(NCORES)], axis=0
    ).astype(np.float32)
    return out, res


def kernel(**inputs):
    nc = _build_program()
    in_maps = _prepare_in_maps(**inputs)
    if "runner" not in _CACHE:
        _CACHE["runner"] = _make_runner(nc, in_maps)
    return _CACHE["runner"](in_maps)



# revision 19
# speedup vs baseline: 1.1957x; 1.1957x over previous
"""Trainium2 Bass kernel for GATRelationNet (self-contained).

Math:
  att_h = attributes @ att_w                        [N, H]
  e     = leaky_relu(att_h@a1 + (att_h@a2).T, 0.2)  [N, N]
  attn  = softmax(e, axis=1)
  att_outs = attn @ att_h                           [N, H]
  img_proj = image_feats @ img_w                    [B, H]
  sem_proj = att_outs @ sem_w + sem_b               [N, H]
  out[b,n] = fc_b + sum_h fc_w[h]*relu(img_proj[b,h] + sem_proj[n,h])

Strategy (8 cores):
  - Replicate the GAT on every core in bf16 (host-rounded operands, no
    on-device rounding pass); shard the relation part over batch (32
    rows/core).
  - |fc_w| is folded into sem_w/img_w/sem_b host-side with a sign/
    permutation trick: h-columns are permuted so that same-sign pairs
    (h, h') occupy the same partition lane in m-chunk pairs (0,1) and
    (2,3).  The relation reduce then needs only +-1 stationary weights,
    so for most batches the four 128-row relu chunks are pair-summed on
    DVE (fp16 tensor_tensor, 2x mode) before the PE reduce - halving
    PE's phase-E column count.
  - relu producers are fp16 tensor_scalar ops hitting DVE's 4x_2p mode
    (0.26 ns/col); work is split ACT/DVE/GPSIMD by tuned ratios.
  - PE stationaries for the reduce are sliding windows of tiny [128,63]
    sign tiles (col 31 = signs) - no per-batch mask DMA.
  - Junk warm-up matmuls during the input DMAs burn the PE p-state ramp
    so real matmuls run at full clock.
"""

import numpy as np
import ml_dtypes

import concourse.bass as bass
import concourse.mybir as mybir
import concourse.tile as tile
from concourse import bacc
from concourse.bass_utils import run_bass_kernel_spmd

P = 128
B, N, A, H, IDIM = 256, 1000, 512, 512, 512
NCORES = 8
BS = B // NCORES      # 32 batch rows per core
KA = A // P           # 4 contraction chunks over A
HM = H // P           # 4 h chunks
NJ = 8                # j (class, softmax-reduced) chunks
JW = N // NJ          # 125
IW = 500              # i half width (PSUM bank = 512 fp32)
NEG = 0.2

# ---- tuning knobs (engine assignment) ----
FB = 17               # batches with DVE-folded reduce (2 PE chunks not 4)
N_ACT = 39            # producer units on ScalarE (of 128)
N_GPS = 27            # producer units on GPSIMD
XB = 6                # phase-B chunks on the DVE variant (rest ScalarE)
SEM2_GPS = 4          # sem2 normalize-copies on GPSIMD (rest DVE)
N_WARM = 8            # PE warm-up matmuls (pre-load)
N_FILL = 3            # PE filler matmuls between f1b k-chunks
# phase-B production order: ACT-variant chunks first so the earliest-
# consumed expT tiles come off the (otherwise idle) ACT engine;
# waves consume expT/att_h in the same order.
JORD = [6, 7, 0, 1, 2, 3, 4, 5]
# folded batches spread evenly over the b loop so DVE (fold) pressure
# interleaves with ACT/GPS-heavy unfolded batches
FOLDED = [b for b in range(BS) if (b + 1) * FB // BS > b * FB // BS]

F32 = mybir.dt.float32
F16 = mybir.dt.float16
BF16 = mybir.dt.bfloat16
AF = mybir.ActivationFunctionType
OP = mybir.AluOpType

_CACHE = {}


def _build_program():
    if "nc" in _CACHE:
        return _CACHE["nc"]

    nc = bacc.Bacc(
        "TRN2", target_bir_lowering=False, debug=False, num_devices=NCORES
    )

    d_attrT = nc.dram_tensor("attrT", [A, N], BF16, kind="ExternalInput")
    d_att_w = nc.dram_tensor("att_w", [P, KA * H], BF16, kind="ExternalInput")
    d_w1b = nc.dram_tensor("w1b", [P, KA * P], BF16, kind="ExternalInput")
    d_w2 = nc.dram_tensor("w2", [P, KA], BF16, kind="ExternalInput")
    d_img_w = nc.dram_tensor("img_w", [P, KA * H], BF16, kind="ExternalInput")
    d_imgfT = nc.dram_tensor("imgfT", [P, KA * BS], BF16, kind="ExternalInput")
    d_sem_w = nc.dram_tensor("sem_w", [P, KA * H], BF16, kind="ExternalInput")
    d_sem_bw = nc.dram_tensor("sem_bw", [P, HM], F32, kind="ExternalInput")
    d_swin = nc.dram_tensor("swin", [P, 6 * 63], F16, kind="ExternalInput")
    d_fc_b = nc.dram_tensor("fc_b", [1, 1], F32, kind="ExternalInput")
    d_out = nc.dram_tensor("out", [BS, N], F32, kind="ExternalOutput")

    with tile.TileContext(nc) as tc:
        _program(
            nc, tc, d_attrT, d_att_w, d_w1b, d_w2, d_img_w, d_imgfT,
            d_sem_w, d_sem_bw, d_swin, d_fc_b, d_out,
        )

    nc.compile()
    _CACHE["nc"] = nc
    return nc


def _producer_engines():
    """Per relu-producer unit -> engine, interleaved so the three
    engines run concurrently (largest-remainder round-robin).  GPSIMD
    (slowest per unit, and the engine gating the final drain) gets no
    units in the last stretch; the last few units go to DVE."""
    total = 128
    counts = {"A": N_ACT, "G": N_GPS, "D": total - N_ACT - N_GPS}
    acc = dict.fromkeys(counts, 0)
    pat = []
    for i in range(total):
        k = max(counts, key=lambda e: counts[e] * (i + 1) - acc[e] * total)
        pat.append(k)
        acc[k] += 1
    # push G out of the tail, pull D in
    tail = total - 12
    for i in range(tail, total):
        if pat[i] == "G":
            for j in range(tail - 1, -1, -1):
                if pat[j] == "D":
                    pat[i], pat[j] = pat[j], pat[i]
                    break
    for i in range(total - 4, total):
        if pat[i] == "A":
            for j in range(total - 5, -1, -1):
                if pat[j] == "D":
                    pat[i], pat[j] = pat[j], pat[i]
                    break
    return pat


def _program(nc, tc, d_attrT, d_att_w, d_w1b, d_w2, d_img_w, d_imgfT,
             d_sem_w, d_sem_bw, d_swin, d_fc_b, d_out):
    cpool_ctx = tc.tile_pool(name="consts", bufs=1)
    cpool = cpool_ctx.__enter__()
    epool_ctx = tc.tile_pool(name="etmp", bufs=2)
    epool = epool_ctx.__enter__()

    # ---- persistent SBUF tiles ----
    attrT = [cpool.tile([P, N], BF16, tag=f"attrT{k}", name=f"attrT{k}")
             for k in range(KA)]
    attwa = cpool.tile([P, KA * H], BF16, tag="attwa", name="attwa")
    att_w = [attwa[:, k * H:(k + 1) * H] for k in range(KA)]
    w1ba = cpool.tile([P, KA * P], BF16, tag="w1ba", name="w1ba")
    w1b = [w1ba[:, k * P:(k + 1) * P] for k in range(KA)]
    w2a = cpool.tile([P, KA], BF16, tag="w2a", name="w2a")
    semwa = cpool.tile([P, KA * H], BF16, tag="semwa", name="semwa")
    sem_w = [semwa[:, k * H:(k + 1) * H] for k in range(KA)]
    imgwa = cpool.tile([P, KA * H], BF16, tag="imgwa", name="imgwa")
    img_w = [imgwa[:, k * H:(k + 1) * H] for k in range(KA)]
    imgfTa = cpool.tile([P, KA * BS], BF16, tag="imgfTa", name="imgfTa")
    sem_bwa = cpool.tile([P, HM], F32, tag="sembwa", name="sembwa")
    swin = cpool.tile([P, 6 * 63], F16, tag="swin", name="swin")
    # windows: [s0, s1, s2, s3, c01, c23]
    win_s = [swin[:, t * 63:(t + 1) * 63] for t in range(4)]
    win_c = [swin[:, (4 + t) * 63:(5 + t) * 63] for t in range(2)]
    fcb = cpool.tile([1, 1], F32, tag="fcb", name="fcb")

    att_h = [cpool.tile([JW, H], BF16, tag=f"atth{j}", name=f"atth{j}")
             for j in range(NJ)]
    expT = [cpool.tile([JW, N], BF16, tag=f"expT{j}", name=f"expT{j}")
            for j in range(NJ)]
    f1b = cpool.tile([P, N], BF16, tag="f1b", name="f1b")
    f2col = [cpool.tile([JW, 1], F32, tag=f"f2col{j}", name=f"f2col{j}")
             for j in range(NJ)]
    imgb = [cpool.tile([P, BS], F32, tag=f"imgb{m}", name=f"imgb{m}")
            for m in range(HM)]
    aoT = [cpool.tile([P, N], BF16, tag=f"aoT{m}", name=f"aoT{m}")
           for m in range(HM)]
    rb_sb = [cpool.tile([P, IW], BF16, tag=f"rb{ih}", name=f"rb{ih}")
             for ih in range(2)]
    sem2T = [cpool.tile([P, N], F16, tag=f"sem2T{m}", name=f"sem2T{m}")
             for m in range(HM)]
    fcb_rep = cpool.tile([BS, 1], F32, tag="fcb_rep", name="fcb_rep")
    out_sb = cpool.tile([BS, N], F32, tag="out_sb", name="out_sb")

    ones_colb = cpool.tile([JW, 1], BF16, tag="ones_colb", name="ones_colb")
    ones_rowb = cpool.tile([1, P], BF16, tag="ones_rowb", name="ones_rowb")
    ones_row = cpool.tile([1, P], F32, tag="ones_row", name="ones_row")
    recip_bf = cpool.tile([1, N], BF16, tag="recip_bf", name="recip_bf")

    junk_st = cpool.tile([P, 2], BF16, tag="junk_st", name="junk_st")
    junk_mv = cpool.tile([P, 512], BF16, tag="junk_mv", name="junk_mv")

    # ---- loads (order matters: earliest-needed first; att_w chunks
    # interleaved with attrT chunks so att_h can start right after) ----
    nc.sync.dma_start(w2a[:], d_w2[:, :])
    nc.sync.dma_start(w1ba[:], d_w1b[:, :])
    for k in range(KA):
        nc.sync.dma_start(attrT[k][:], d_attrT[k * P:(k + 1) * P, :])
        ksl = slice(k * H, (k + 1) * H)
        nc.sync.dma_start(attwa[:, ksl], d_att_w[:, ksl])
    nc.sync.dma_start(semwa[:], d_sem_w[:, :])
    nc.sync.dma_start(imgwa[:], d_img_w[:, :])
    nc.sync.dma_start(imgfTa[:], d_imgfT[:, :])
    nc.sync.dma_start(sem_bwa[:], d_sem_bw[:, :])
    nc.sync.dma_start(swin[:], d_swin[:, :])
    nc.sync.dma_start(fcb[:], d_fc_b[:, :])

    nc.vector.memset(junk_st[:], 0.0)
    nc.vector.memset(junk_mv[:], 0.0)
    nc.vector.memset(ones_colb[:], 1.0)
    nc.vector.memset(ones_rowb[:], 1.0)
    nc.vector.memset(ones_row[:], 1.0)

    # warm up the gpsimd ucode ops early (op load is ~us)
    gps_warm = cpool.tile([P, 8], F16, tag="gpswarm", name="gpswarm")
    nc.gpsimd.memset(gps_warm[:], 0.0)
    nc.gpsimd.tensor_scalar(
        gps_warm[:], gps_warm[:], 0.0, 0.0, op0=OP.add, op1=OP.max
    )

    # ---- phase A: f1b (k-outer, PE fillers between chunks), f2 ----
    psumA1_ctx = tc.tile_pool(name="psumA1", bufs=1, space="PSUM")
    psumA1 = psumA1_ctx.__enter__()
    ps_w = psumA1.tile([2, 512], F32, tag="warm", name="warm")
    for _ in range(N_WARM):
        nc.tensor.matmul(ps_w[:], junk_st[:], junk_mv[:],
                         start=True, stop=True)

    # f1b [128, 1000]: stationary w1-broadcast chunks, k-outer so each
    # attrT chunk is consumed as its DMA lands; junk fillers keep PE
    # busy (p-state) while the next chunk loads.
    ps_f1 = [
        psumA1.tile([P, IW], F32, tag=f"f1b{ih}", name=f"f1b{ih}")
        for ih in range(2)
    ]
    for k in range(KA):
        for ih in range(2):
            isl = slice(ih * IW, (ih + 1) * IW)
            nc.tensor.matmul(
                ps_f1[ih][:], w1b[k][:], attrT[k][:, isl],
                start=(k == 0), stop=(k == KA - 1),
            )
        if k < KA - 1:
            for _ in range(N_FILL):
                nc.tensor.matmul(ps_w[:], junk_st[:], junk_mv[:],
                                 start=True, stop=True)
    for ih in range(2):
        nc.vector.tensor_copy(f1b[:, ih * IW:(ih + 1) * IW], ps_f1[ih][:])

    # f2 column per j chunk: [125, 1] accumulated over k
    for j in range(NJ):
        ps = psumA1.tile([JW, 1], F32, tag="f2", name="f2", bufs=2)
        jsl = slice(j * JW, (j + 1) * JW)
        for k in range(KA):
            nc.tensor.matmul(
                ps[:], attrT[k][:, jsl], w2a[:, k:k + 1],
                start=(k == 0), stop=(k == KA - 1),
            )
        nc.vector.tensor_copy(f2col[j][:], ps[:])

    psumA2_ctx = tc.tile_pool(name="psumA2", bufs=1, space="PSUM")
    psumA2 = psumA2_ctx.__enter__()

    # ---- phase B: e -> leaky -> exp, in JORD production order ----
    for j in JORD:
        if j >= XB:
            # ACT path: Prelu (leaky) with f2 bias, then Exp
            e_t = epool.tile([JW, N], BF16, tag="e", name="e")
            nc.scalar.activation(
                e_t[:], f1b[0:JW, :], AF.Prelu, bias=f2col[j][:, 0:1],
                alpha=NEG,
            )
            nc.scalar.activation(expT[j][:], e_t[:], AF.Exp)
        else:
            # DVE path: e = f1+f2, e02 = 0.2*(f1+f2), max, then Exp
            e_t = epool.tile([JW, N], BF16, tag="e", name="e")
            e2 = epool.tile([JW, N], BF16, tag="e2", name="e2")
            nc.vector.tensor_scalar(
                e_t[:], f1b[0:JW, :], f2col[j][:, 0:1], None, op0=OP.add
            )
            nc.vector.tensor_scalar(
                e2[:], f1b[0:JW, :], f2col[j][:, 0:1], NEG,
                op0=OP.add, op1=OP.mult,
            )
            nc.vector.tensor_tensor(e_t[:], e_t[:], e2[:], op=OP.max)
            nc.scalar.activation(expT[j][:], e_t[:], AF.Exp)

    # att_h natural [j, h] (lhsT for the ao matmul); copies alternate
    # ACT/DVE (GPSIMD cannot read PSUM)
    for ji, j in enumerate(JORD):
        ps = psumA2.tile([JW, H], F32, tag="ah", name="ah", bufs=3)
        jsl = slice(j * JW, (j + 1) * JW)
        for k in range(KA):
            nc.tensor.matmul(
                ps[:], attrT[k][:, jsl], att_w[k][:],
                start=(k == 0), stop=(k == KA - 1),
            )
        if ji % 2 == 0:
            nc.scalar.copy(att_h[j][:], ps[:])
        else:
            nc.vector.tensor_copy(att_h[j][:], ps[:])

    psumA2_ctx.__exit__(None, None, None)
    psumA1_ctx.__exit__(None, None, None)

    # ---- phase C: colsum + unnormalized att_outs^T, in two ih waves
    # with j innermost so expT[j] chunks are consumed as phase B
    # produces them ----
    with tc.tile_pool(name="psumB", bufs=1, space="PSUM") as psumB:
        cs_row = epool.tile([1, N], F32, tag="cs_row", name="cs_row")
        ps_cs = [
            psumB.tile([1, IW], F32, tag=f"cs{ih}", name=f"cs{ih}")
            for ih in range(2)
        ]
        ps_ao = [
            psumB.tile([P, IW], F32, tag=f"ao{m}", name=f"ao{m}")
            for m in range(HM)
        ]
        # wave A (ih=0): colsum (both halves) + ao half 0, JORD order
        for ji, j in enumerate(JORD):
            for ih in range(2):
                isl = slice(ih * IW, (ih + 1) * IW)
                nc.tensor.matmul(
                    ps_cs[ih][:], ones_colb[:], expT[j][:, isl],
                    start=(ji == 0), stop=(ji == NJ - 1),
                )
            for m in range(HM):
                nc.tensor.matmul(
                    ps_ao[m][:], att_h[j][:, m * P:(m + 1) * P],
                    expT[j][:, 0:IW],
                    start=(ji == 0), stop=(ji == NJ - 1),
                )
        for ih in range(2):
            nc.vector.tensor_copy(
                cs_row[:, ih * IW:(ih + 1) * IW], ps_cs[ih][:]
            )
        recip_f = epool.tile([1, N], F32, tag="recip_f", name="recip_f")
        rc_scr = epool.tile([1, N], F32, tag="rc_scr", name="rc_scr")
        nc.vector.reciprocal_approx_accurate(
            out=recip_f[:], in_=cs_row[:], scratch=rc_scr[:]
        )
        nc.vector.tensor_copy(recip_bf[:], recip_f[:])
        for m in range(HM):
            if m % 2 == 0:
                nc.scalar.copy(aoT[m][:, 0:IW], ps_ao[m][:])
            else:
                nc.vector.tensor_copy(aoT[m][:, 0:IW], ps_ao[m][:])
        # wave B (ih=1); junk fillers cover the bank-release wait so the
        # PE p-state never resets
        ps_w2 = psumB.tile([2, 512], F32, tag="warm2", name="warm2")
        for _ in range(N_FILL):
            nc.tensor.matmul(ps_w2[:], junk_st[:], junk_mv[:],
                             start=True, stop=True)
        ps_ao2 = [
            psumB.tile([P, IW], F32, tag=f"ao{m}", name=f"ao{m}b")
            for m in range(HM)
        ]
        for ji, j in enumerate(JORD):
            for m in range(HM):
                nc.tensor.matmul(
                    ps_ao2[m][:], att_h[j][:, m * P:(m + 1) * P],
                    expT[j][:, IW:N],
                    start=(ji == 0), stop=(ji == NJ - 1),
                )
        for m in range(HM):
            if m % 2 == 0:
                nc.scalar.copy(aoT[m][:, IW:N], ps_ao2[m][:])
            else:
                nc.vector.tensor_copy(aoT[m][:, IW:N], ps_ao2[m][:])
        # broadcast recip row to 128 partitions (bf16)
        for ih in range(2):
            isl = slice(ih * IW, (ih + 1) * IW)
            ps_rb = psumB.tile([P, IW], F32, tag=f"cs{ih}", name=f"rbp{ih}")
            nc.tensor.matmul(ps_rb[:], ones_rowb[:], recip_bf[:, isl])
            nc.vector.tensor_copy(rb_sb[ih][:], ps_rb[:])
        # fillers cover the psumB bank drain so img/sem2 dispatch at
        # full p-state
        for _ in range(2 * N_FILL):
            nc.tensor.matmul(ps_w2[:], junk_st[:], junk_mv[:],
                             start=True, stop=True)

    # ---- phase A2: img_proj (|w|-scaled via img_w) + sem_b fold ----
    with tc.tile_pool(name="psumI", bufs=1, space="PSUM") as psumI:
        for m in range(HM):
            ps = psumI.tile([P, BS], F32, tag="img", name="img", bufs=2)
            msl = slice(m * P, (m + 1) * P)
            for k in range(KA):
                nc.tensor.matmul(
                    ps[:], img_w[k][:, msl], imgfTa[:, k * BS:(k + 1) * BS],
                    start=(k == 0), stop=(k == KA - 1),
                )
            nc.scalar.activation(
                imgb[m][:], ps[:], AF.Identity, bias=sem_bwa[:, m:m + 1]
            )
        ps = psumI.tile([BS, 1], F32, tag="fcbp", name="fcbp")
        nc.tensor.matmul(ps[:], ones_row[0:1, 0:BS], fcb[0:1, 0:1])
        nc.vector.tensor_copy(fcb_rep[:], ps[:])

    # ---- phase D + E interleaved by m-chunk pair ----
    epool_ctx.__exit__(None, None, None)
    rpool_ctx = tc.tile_pool(name="relu", bufs=16)
    rpool = rpool_ctx.__enter__()
    zpool_ctx = tc.tile_pool(name="zfold", bufs=8)
    zpool = zpool_ctx.__enter__()

    pat = _producer_engines()
    pi = 0

    def producer(dst, m, b):
        nonlocal pi
        eng = pat[pi % len(pat)]
        pi += 1
        bias = imgb[m][:, b:b + 1]
        if eng == "A":
            nc.scalar.activation(dst[:], sem2T[m][:], AF.Relu, bias=bias)
        elif eng == "D":
            nc.vector.tensor_scalar(
                dst[:], sem2T[m][:], bias, 0.0, op0=OP.add, op1=OP.max
            )
        else:
            nc.gpsimd.tensor_scalar(
                dst[:], sem2T[m][:], bias, 0.0, op0=OP.add, op1=OP.max
            )

    psumC_ctx = tc.tile_pool(name="psumC", bufs=1, space="PSUM")
    psumC = psumC_ctx.__enter__()
    psumD_ctx = tc.tile_pool(name="psumD", bufs=1, space="PSUM")
    psumD = psumD_ctx.__enter__()
    out_ps = [
        psumD.tile([BS, IW], F32, tag=f"out{ih}", name=f"out{ih}")
        for ih in range(2)
    ]

    sem2_cnt = [0]

    def sem2_chunk(m):
        """sem2T'[m] = (sem_w'^T @ aoT) * recip  (fp16 out)."""
        msl = slice(m * P, (m + 1) * P)
        for ih in range(2):
            isl = slice(ih * IW, (ih + 1) * IW)
            ps = psumC.tile([P, IW], F32, tag="s2", name="s2", bufs=3)
            for k in range(KA):
                nc.tensor.matmul(
                    ps[:], sem_w[k][:, msl], aoT[k][:, isl],
                    start=(k == 0), stop=(k == KA - 1),
                )
            nc.vector.tensor_tensor(
                sem2T[m][:, isl], ps[:], rb_sb[ih][:], op=OP.mult
            )
            sem2_cnt[0] += 1

    # moving-operand count per ih: folded b -> 1, unfolded -> 2 per group
    n_mv = 2 * (FB + 2 * (BS - FB))
    mv_idx = [0]

    def e_matmul(stat_win, b, mv):
        """One reduce matmul pair into out_ps (accumulating)."""
        for ih in range(2):
            isl = slice(ih * IW, (ih + 1) * IW)
            nc.tensor.matmul(
                out_ps[ih][:], stat_win[:, 31 - b:63 - b], mv[:, isl],
                start=(mv_idx[0] == 0), stop=(mv_idx[0] == n_mv - 1),
            )
        mv_idx[0] += 1

    def phase_e_group(q):
        c0, c1 = 2 * q, 2 * q + 1
        for b in range(BS):
            if b in FOLDED:
                r0 = rpool.tile([P, N], F16, tag="r", name="r")
                r1 = rpool.tile([P, N], F16, tag="r", name="r")
                producer(r0, c0, b)
                producer(r1, c1, b)
                z = zpool.tile([P, N], F16, tag="z", name="z")
                nc.vector.tensor_tensor(z[:], r0[:], r1[:], op=OP.add)
                e_matmul(win_c[q], b, z)
            else:
                for c in (c0, c1):
                    r = rpool.tile([P, N], F16, tag="r", name="r")
                    producer(r, c, b)
                    e_matmul(win_s[c], b, r)

    for m in range(HM):
        sem2_chunk(m)
    phase_e_group(0)
    phase_e_group(1)

    nc.vector.tensor_scalar(
        out_sb[:, 0:IW], out_ps[0][:], fcb_rep[:, 0:1], None, op0=OP.add
    )
    nc.scalar.activation(
        out_sb[:, IW:N], out_ps[1][:], AF.Identity, bias=fcb_rep[:, 0:1],
    )
    nc.sync.dma_start(d_out[:, :], out_sb[:])

    psumD_ctx.__exit__(None, None, None)
    psumC_ctx.__exit__(None, None, None)
    zpool_ctx.__exit__(None, None, None)
    rpool_ctx.__exit__(None, None, None)
    cpool_ctx.__exit__(None, None, None)


def _prepare_in_maps(image_feats, attributes, att_w, att_a, img_w, sem_w,
                     sem_b, fc_w, fc_b):
    f = np.float32
    bf = ml_dtypes.bfloat16
    attributes = np.asarray(attributes, f)
    att_w = np.asarray(att_w, f)
    att_a = np.asarray(att_a, f)
    image_feats = np.asarray(image_feats, f)
    sem_w = np.asarray(sem_w, f)
    img_w = np.asarray(img_w, f)
    sem_b = np.asarray(sem_b, f).reshape(H)
    fc_w = np.asarray(fc_w, f).reshape(H)
    fc_b = np.asarray(fc_b, f).reshape(1, 1)

    attrT = np.ascontiguousarray(attributes.T).astype(bf)       # [A, N]
    a1, a2 = att_a[:H, 0], att_a[H:, 0]
    w1 = (att_w @ a1).astype(f)                                 # [A]
    w2 = (att_w @ a2).astype(f)                                 # [A]
    # w1 broadcast chunks: w1b[k][a, p] = w1[k*128+a] for all p
    w1b = np.repeat(
        w1.reshape(KA, P, 1), P, axis=2
    ).transpose(1, 0, 2).reshape(P, KA * P).astype(bf)
    w1b = np.ascontiguousarray(w1b)
    w2p = np.ascontiguousarray(
        w2.reshape(KA, P).T
    ).astype(bf)                                                # [128, KA]

    # ---- sign/permutation machinery for the relation reduce ----
    w = fc_w.astype(np.float64).copy()
    sg = np.sign(w)
    if (sg > 0).sum() % 2 == 1:
        w[np.argmin(np.abs(w))] = 0.0
        sg = np.sign(w)
    pos = list(np.where(sg > 0)[0])
    neg = list(np.where(sg < 0)[0])
    wc = list(np.where(sg == 0)[0])  # 0 or 1 wildcards
    couples = []
    csigns = []
    for lst, s in ((pos, 1.0), (neg, -1.0)):
        while len(lst) >= 2:
            couples.append((lst.pop(), lst.pop()))
            csigns.append(s)
        if len(lst) == 1:
            couples.append((lst.pop(), wc.pop()))
            csigns.append(s)
    while len(couples) < 2 * P:  # only if many zero weights
        couples.append((wc.pop(), wc.pop()))
        csigns.append(0.0)
    assert len(couples) == 2 * P, len(couples)

    h_ord = np.zeros((HM, P), np.int64)
    s_chunk = np.zeros((HM, P), f)
    c_sign = np.zeros((2, P), f)
    for k, ((ha, hb), s) in enumerate(zip(couples, csigns)):
        q, p = k // P, k % P
        h_ord[2 * q][p] = ha
        h_ord[2 * q + 1][p] = hb
        s_chunk[2 * q][p] = sg[ha] if sg[ha] != 0 else 0.0
        s_chunk[2 * q + 1][p] = sg[hb] if sg[hb] != 0 else 0.0
        c_sign[q][p] = s
    perm = h_ord.reshape(H)
    aw = np.abs(w).astype(f)[perm]                              # |w| permuted

    # fold |w| into sem_w / img_w columns (permuted), sem_b
    sem_wp = (sem_w[:, perm] * aw[None, :]).astype(bf)
    img_wp = (img_w[:, perm] * aw[None, :]).astype(bf)
    sem_bw = (sem_b[perm] * aw).reshape(HM, P).T.astype(f)      # [128, HM]
    sem_bw = np.ascontiguousarray(sem_bw)

    # sign windows [128, 6*63]: col 31 of each window = signs
    swin = np.zeros((P, 6, 63), f)
    for c in range(4):
        swin[:, c, 31] = s_chunk[c]
    swin[:, 4, 31] = c_sign[0]
    swin[:, 5, 31] = c_sign[1]
    swin = np.ascontiguousarray(
        swin.reshape(P, 6 * 63).astype(np.float16)
    )

    def pack_k(wm):
        return np.ascontiguousarray(
            np.asarray(wm, bf).reshape(KA, P, H).transpose(1, 0, 2)
            .reshape(P, KA * H)
        )
    att_w_packed = pack_k(att_w.astype(bf))
    sem_w_packed = pack_k(sem_wp)
    img_w_packed = pack_k(img_wp)

    shared = {
        "attrT": attrT, "att_w": att_w_packed, "w1b": w1b, "w2": w2p,
        "img_w": img_w_packed, "sem_w": sem_w_packed, "sem_bw": sem_bw,
        "swin": swin, "fc_b": fc_b,
    }
    in_maps = []
    for c in range(NCORES):
        imgfT = np.ascontiguousarray(
            image_feats[c * BS:(c + 1) * BS, :].T
            .reshape(KA, P, BS).transpose(1, 0, 2).reshape(P, KA * BS)
        ).astype(bf)
        in_maps.append(dict(shared, imgfT=imgfT))
    return in_maps


def _make_runner(nc, in_maps):
    """Build the sharded PJRT callable once (mirrors
    bass2jax.run_bass_via_pjrt's multi-core path) so repeated kernel()
    calls reuse the compiled NEFF executable."""
    import jax
    from jax.sharding import Mesh, PartitionSpec

    try:
        from jax.experimental.shard_map import shard_map
    except ImportError:
        shard_map = jax.shard_map
    from concourse import bass2jax

    bass2jax.install_neuronx_cc_hook()
    n_cores = len(in_maps)
    partition_name = (
        nc.partition_id_tensor.name if nc.partition_id_tensor else None
    )
    in_names, out_names, out_avals = [], [], []
    for alloc in nc.m.functions[0].allocations:
        if not isinstance(alloc, mybir.MemoryLocationSet):
            continue
        name = alloc.memorylocations[0].name
        if alloc.kind == "ExternalInput":
            if name != partition_name:
                in_names.append(name)
        elif alloc.kind == "ExternalOutput":
            out_names.append(name)
            out_avals.append(
                jax.core.ShapedArray(
                    tuple(alloc.tensor_shape), mybir.dt.np(alloc.dtype)
                )
            )
    all_in_names = list(in_names) + list(out_names)
    if partition_name is not None:
        all_in_names.append(partition_name)
    n_params, n_outs = len(in_names), len(out_avals)

    def _body(*args):
        operands = list(args)
        if partition_name is not None:
            operands.append(bass2jax.partition_id_tensor())
        return tuple(bass2jax._bass_exec_p.bind(
            *operands,
            out_avals=tuple(out_avals),
            in_names=tuple(all_in_names),
            out_names=tuple(out_names),
            lowering_input_output_aliases=(),
            sim_require_finite=True,
            sim_require_nnan=True,
            nc=nc,
        ))

    donate = tuple(range(n_params, n_params + n_outs))
    devices = jax.devices()[:n_cores]
    mesh = Mesh(np.asarray(devices), ("core",))
    sharded = jax.jit(
        shard_map(
            _body, mesh=mesh,
            in_specs=(PartitionSpec("core"),) * (n_params + n_outs),
            out_specs=(PartitionSpec("core"),) * n_outs,
            check_rep=False,
        ),
        donate_argnums=donate, keep_unused=True,
    )

    import zlib

    def call(maps):
        concat_in = [
            np.concatenate([np.asarray(maps[c][n]) for c in range(n_cores)], 0)
            for n in in_names
        ]
        key = tuple(zlib.adler32(x.tobytes()) for x in concat_in)
        dev = _CACHE.get("dev_inputs")
        if dev is None or dev[0] != key:
            dev = (key, [jax.device_put(x) for x in concat_in])
            _CACHE["dev_inputs"] = dev
        zeros = [
            np.zeros((n_cores * av.shape[0], *av.shape[1:]), av.dtype)
            for av in out_avals
        ]
        outs = sharded(*dev[1], *zeros)
        jax.block_until_ready(outs)
        oi = out_names.index("out")
        full = np.asarray(outs[oi]).reshape(n_cores, *out_avals[oi].shape)
        return np.concatenate(list(full), axis=0).astype(np.float32)

    return call


def run(inputs, **spmd_kwargs):
    """Returns (full output [B, N], BassKernelResults) via the generic
    run_bass_kernel_spmd path (used by test tooling)."""
    nc = _build_program()
    in_maps = _prepare_in_maps(**inputs)
    res = run_bass_kernel_spmd(nc, in_maps, list(range(NCORES)), **spmd_kwargs)
    out = np.concatenate(
        [res.results[c]["out"] for c in range(NCORES)], axis=0
    ).astype(np.float32)
    return out, res


def kernel(**inputs):
    nc = _build_program()
    in_maps = _prepare_in_maps(**inputs)
    if "runner" not in _CACHE:
        _CACHE["runner"] = _make_runner(nc, in_maps)
    return _CACHE["runner"](in_maps)


# revision 23
# speedup vs baseline: 1.2195x; 1.0199x over previous
"""Trainium2 Bass kernel for GATRelationNet (self-contained).

Math:
  att_h = attributes @ att_w                        [N, H]
  e     = leaky_relu(att_h@a1 + (att_h@a2).T, 0.2)  [N, N]
  attn  = softmax(e, axis=1)
  att_outs = attn @ att_h                           [N, H]
  img_proj = image_feats @ img_w                    [B, H]
  sem_proj = att_outs @ sem_w + sem_b               [N, H]
  out[b,n] = fc_b + sum_h fc_w[h]*relu(img_proj[b,h] + sem_proj[n,h])

Strategy (8 cores):
  - Replicate the GAT on every core in bf16 (host-rounded operands, no
    on-device rounding pass); shard the relation part over batch (32
    rows/core).
  - |fc_w| is folded into sem_w/img_w/sem_b host-side with a sign/
    permutation trick: h-columns are permuted so that same-sign pairs
    (h, h') occupy the same partition lane in m-chunk pairs (0,1) and
    (2,3).  The relation reduce then needs only +-1 stationary weights,
    so for most batches the four 128-row relu chunks are pair-summed on
    DVE (fp16 tensor_tensor, 2x mode) before the PE reduce - halving
    PE's phase-E column count.
  - relu producers are fp16 tensor_scalar ops hitting DVE's 4x_2p mode
    (0.26 ns/col); work is split ACT/DVE/GPSIMD by tuned ratios.
  - PE stationaries for the reduce are sliding windows of tiny [128,63]
    sign tiles (col 31 = signs) - no per-batch mask DMA.
  - Junk warm-up matmuls during the input DMAs burn the PE p-state ramp
    so real matmuls run at full clock.
"""

import numpy as np
import ml_dtypes

import concourse.bass as bass
import concourse.mybir as mybir
import concourse.tile as tile
from concourse import bacc
from concourse.bass_utils import run_bass_kernel_spmd

P = 128
B, N, A, H, IDIM = 256, 1000, 512, 512, 512
NCORES = 8
BS = B // NCORES      # 32 batch rows per core
KA = A // P           # 4 contraction chunks over A
HM = H // P           # 4 h chunks
NJ = 8                # j (class, softmax-reduced) chunks
JW = N // NJ          # 125
IW = 500              # i half width (PSUM bank = 512 fp32)
NEG = 0.2

# ---- tuning knobs (engine assignment) ----
FB = 17               # batches with DVE-folded reduce (2 PE chunks not 4)
N_ACT = 40            # producer units on ScalarE (of 128)
N_GPS = 24            # producer units on GPSIMD
XB = 4                # phase-B chunks on the DVE variant (rest ScalarE)
SEM2_GPS = 4          # sem2 normalize-copies on GPSIMD (rest DVE)
N_WARM = 8            # PE warm-up matmuls (pre-load)
N_FILL = 2            # PE filler matmuls between f1b k-chunks
# phase-B production order: ACT-variant chunks first so the earliest-
# consumed expT tiles come off the (otherwise idle) ACT engine;
# waves consume expT/att_h in the same order.
JORD = [6, 7, 0, 1, 2, 3, 4, 5]
# folded batches spread evenly over the b loop so DVE (fold) pressure
# interleaves with ACT/GPS-heavy unfolded batches
FOLDED = [b for b in range(BS) if (b + 1) * FB // BS > b * FB // BS]

F32 = mybir.dt.float32
F16 = mybir.dt.float16
BF16 = mybir.dt.bfloat16
AF = mybir.ActivationFunctionType
OP = mybir.AluOpType

_CACHE = {}


def _build_program():
    if "nc" in _CACHE:
        return _CACHE["nc"]

    nc = bacc.Bacc(
        "TRN2", target_bir_lowering=False, debug=False, num_devices=NCORES
    )

    d_attrT = nc.dram_tensor("attrT", [A, N], BF16, kind="ExternalInput")
    d_att_w = nc.dram_tensor("att_w", [P, KA * H], BF16, kind="ExternalInput")
    d_w1b = nc.dram_tensor("w1b", [P, KA * P], BF16, kind="ExternalInput")
    d_w2 = nc.dram_tensor("w2", [P, KA], BF16, kind="ExternalInput")
    d_img_w = nc.dram_tensor("img_w", [P, KA * H], BF16, kind="ExternalInput")
    d_imgfT = nc.dram_tensor("imgfT", [P, KA * BS], BF16, kind="ExternalInput")
    d_sem_w = nc.dram_tensor("sem_w", [P, KA * H], BF16, kind="ExternalInput")
    d_sem_bw = nc.dram_tensor("sem_bw", [P, HM], F32, kind="ExternalInput")
    d_swin = nc.dram_tensor("swin", [P, 6 * 63], F16, kind="ExternalInput")
    d_fc_b = nc.dram_tensor("fc_b", [1, 1], F32, kind="ExternalInput")
    d_out = nc.dram_tensor("out", [BS, N], F32, kind="ExternalOutput")

    with tile.TileContext(nc) as tc:
        _program(
            nc, tc, d_attrT, d_att_w, d_w1b, d_w2, d_img_w, d_imgfT,
            d_sem_w, d_sem_bw, d_swin, d_fc_b, d_out,
        )

    nc.compile()
    _CACHE["nc"] = nc
    return nc


def _producer_engines():
    """Per relu-producer unit -> engine, interleaved so the three
    engines run concurrently (largest-remainder round-robin).  GPSIMD
    (slowest per unit, and the engine gating the final drain) gets no
    units in the last stretch; the last few units go to DVE."""
    total = 128
    counts = {"A": N_ACT, "G": N_GPS, "D": total - N_ACT - N_GPS}
    acc = dict.fromkeys(counts, 0)
    pat = []
    for i in range(total):
        k = max(counts, key=lambda e: counts[e] * (i + 1) - acc[e] * total)
        pat.append(k)
        acc[k] += 1
    # push G out of the tail, pull D in
    tail = total - 12
    for i in range(tail, total):
        if pat[i] == "G":
            for j in range(tail - 1, -1, -1):
                if pat[j] == "D":
                    pat[i], pat[j] = pat[j], pat[i]
                    break
    for i in range(total - 4, total):
        if pat[i] == "A":
            for j in range(total - 5, -1, -1):
                if pat[j] == "D":
                    pat[i], pat[j] = pat[j], pat[i]
                    break
    return pat


def _program(nc, tc, d_attrT, d_att_w, d_w1b, d_w2, d_img_w, d_imgfT,
             d_sem_w, d_sem_bw, d_swin, d_fc_b, d_out):
    cpool_ctx = tc.tile_pool(name="consts", bufs=1)
    cpool = cpool_ctx.__enter__()
    epool_ctx = tc.tile_pool(name="etmp", bufs=2)
    epool = epool_ctx.__enter__()

    # ---- persistent SBUF tiles ----
    attrT = [cpool.tile([P, N], BF16, tag=f"attrT{k}", name=f"attrT{k}")
             for k in range(KA)]
    attwa = cpool.tile([P, KA * H], BF16, tag="attwa", name="attwa")
    att_w = [attwa[:, k * H:(k + 1) * H] for k in range(KA)]
    w1ba = cpool.tile([P, KA * P], BF16, tag="w1ba", name="w1ba")
    w1b = [w1ba[:, k * P:(k + 1) * P] for k in range(KA)]
    w2a = cpool.tile([P, KA], BF16, tag="w2a", name="w2a")
    semwa = cpool.tile([P, KA * H], BF16, tag="semwa", name="semwa")
    sem_w = [semwa[:, k * H:(k + 1) * H] for k in range(KA)]
    imgwa = cpool.tile([P, KA * H], BF16, tag="imgwa", name="imgwa")
    img_w = [imgwa[:, k * H:(k + 1) * H] for k in range(KA)]
    imgfTa = cpool.tile([P, KA * BS], BF16, tag="imgfTa", name="imgfTa")
    sem_bwa = cpool.tile([P, HM], F32, tag="sembwa", name="sembwa")
    swin = cpool.tile([P, 6 * 63], F16, tag="swin", name="swin")
    # windows: [s0, s1, s2, s3, c01, c23]
    win_s = [swin[:, t * 63:(t + 1) * 63] for t in range(4)]
    win_c = [swin[:, (4 + t) * 63:(5 + t) * 63] for t in range(2)]
    fcb = cpool.tile([1, 1], F32, tag="fcb", name="fcb")

    att_h = [cpool.tile([JW, H], BF16, tag=f"atth{j}", name=f"atth{j}")
             for j in range(NJ)]
    expT = [cpool.tile([JW, N], BF16, tag=f"expT{j}", name=f"expT{j}")
            for j in range(NJ)]
    f1b = cpool.tile([P, N], BF16, tag="f1b", name="f1b")
    f2col = [cpool.tile([JW, 1], F32, tag=f"f2col{j}", name=f"f2col{j}")
             for j in range(NJ)]
    imgb = [cpool.tile([P, BS], F32, tag=f"imgb{m}", name=f"imgb{m}")
            for m in range(HM)]
    aoT = [cpool.tile([P, N], BF16, tag=f"aoT{m}", name=f"aoT{m}")
           for m in range(HM)]
    rb_sb = [cpool.tile([P, IW], BF16, tag=f"rb{ih}", name=f"rb{ih}")
             for ih in range(2)]
    sem2T = [cpool.tile([P, N], F16, tag=f"sem2T{m}", name=f"sem2T{m}")
             for m in range(HM)]
    fcb_rep = cpool.tile([BS, 1], F32, tag="fcb_rep", name="fcb_rep")
    out_sb = cpool.tile([BS, N], F32, tag="out_sb", name="out_sb")

    ones_colb = cpool.tile([JW, 1], BF16, tag="ones_colb", name="ones_colb")
    ones_rowb = cpool.tile([1, P], BF16, tag="ones_rowb", name="ones_rowb")
    ones_row = cpool.tile([1, P], F32, tag="ones_row", name="ones_row")
    recip_bf = cpool.tile([1, N], BF16, tag="recip_bf", name="recip_bf")

    junk_st = cpool.tile([P, 2], BF16, tag="junk_st", name="junk_st")
    junk_mv = cpool.tile([P, 512], BF16, tag="junk_mv", name="junk_mv")

    # ---- loads (order matters: earliest-needed first; att_w chunks
    # interleaved with attrT chunks so att_h can start right after) ----
    nc.sync.dma_start(w2a[:], d_w2[:, :])
    nc.sync.dma_start(w1ba[:], d_w1b[:, :])
    for k in range(KA):
        nc.sync.dma_start(attrT[k][:], d_attrT[k * P:(k + 1) * P, :])
        ksl = slice(k * H, (k + 1) * H)
        nc.sync.dma_start(attwa[:, ksl], d_att_w[:, ksl])
    nc.sync.dma_start(semwa[:], d_sem_w[:, :])
    nc.sync.dma_start(imgwa[:], d_img_w[:, :])
    nc.sync.dma_start(imgfTa[:], d_imgfT[:, :])
    nc.sync.dma_start(sem_bwa[:], d_sem_bw[:, :])
    nc.sync.dma_start(swin[:], d_swin[:, :])
    nc.sync.dma_start(fcb[:], d_fc_b[:, :])

    nc.vector.memset(junk_st[:], 0.0)
    nc.vector.memset(junk_mv[:], 0.0)
    nc.vector.memset(ones_colb[:], 1.0)
    nc.vector.memset(ones_rowb[:], 1.0)
    nc.vector.memset(ones_row[:], 1.0)

    # warm up the gpsimd ucode ops early (op load is ~us)
    gps_warm = cpool.tile([P, 8], F16, tag="gpswarm", name="gpswarm")
    nc.gpsimd.memset(gps_warm[:], 0.0)
    nc.gpsimd.tensor_scalar(
        gps_warm[:], gps_warm[:], 0.0, 0.0, op0=OP.add, op1=OP.max
    )

    # ---- phase A: f1b (k-outer, PE fillers between chunks), f2 ----
    psumA1_ctx = tc.tile_pool(name="psumA1", bufs=1, space="PSUM")
    psumA1 = psumA1_ctx.__enter__()
    ps_w = psumA1.tile([2, 512], F32, tag="warm", name="warm")
    for _ in range(N_WARM):
        nc.tensor.matmul(ps_w[:], junk_st[:], junk_mv[:],
                         start=True, stop=True)

    # f1b [128, 1000]: stationary w1-broadcast chunks, k-outer so each
    # attrT chunk is consumed as its DMA lands; junk fillers keep PE
    # busy (p-state) while the next chunk loads.
    ps_f1 = [
        psumA1.tile([P, IW], F32, tag=f"f1b{ih}", name=f"f1b{ih}")
        for ih in range(2)
    ]
    for k in range(KA):
        for ih in range(2):
            isl = slice(ih * IW, (ih + 1) * IW)
            nc.tensor.matmul(
                ps_f1[ih][:], w1b[k][:], attrT[k][:, isl],
                start=(k == 0), stop=(k == KA - 1),
            )
        if k < KA - 1:
            for _ in range(N_FILL):
                nc.tensor.matmul(ps_w[:], junk_st[:], junk_mv[:],
                                 start=True, stop=True)
    for ih in range(2):
        nc.vector.tensor_copy(f1b[:, ih * IW:(ih + 1) * IW], ps_f1[ih][:])

    # f2 column per j chunk: [125, 1] accumulated over k
    for j in range(NJ):
        ps = psumA1.tile([JW, 1], F32, tag="f2", name="f2", bufs=2)
        jsl = slice(j * JW, (j + 1) * JW)
        for k in range(KA):
            nc.tensor.matmul(
                ps[:], attrT[k][:, jsl], w2a[:, k:k + 1],
                start=(k == 0), stop=(k == KA - 1),
            )
        nc.vector.tensor_copy(f2col[j][:], ps[:])

    psumA2_ctx = tc.tile_pool(name="psumA2", bufs=1, space="PSUM")
    psumA2 = psumA2_ctx.__enter__()

    # ---- phase B: e -> leaky -> exp, in JORD production order ----
    for j in JORD:
        if j >= XB:
            # ACT path: Prelu (leaky) with f2 bias, then Exp
            e_t = epool.tile([JW, N], BF16, tag="e", name="e")
            nc.scalar.activation(
                e_t[:], f1b[0:JW, :], AF.Prelu, bias=f2col[j][:, 0:1],
                alpha=NEG,
            )
            nc.scalar.activation(expT[j][:], e_t[:], AF.Exp)
        else:
            # DVE path: e = f1+f2, e02 = 0.2*(f1+f2), max, then Exp
            e_t = epool.tile([JW, N], BF16, tag="e", name="e")
            e2 = epool.tile([JW, N], BF16, tag="e2", name="e2")
            nc.vector.tensor_scalar(
                e_t[:], f1b[0:JW, :], f2col[j][:, 0:1], None, op0=OP.add
            )
            nc.vector.tensor_scalar(
                e2[:], f1b[0:JW, :], f2col[j][:, 0:1], NEG,
                op0=OP.add, op1=OP.mult,
            )
            nc.vector.tensor_tensor(e_t[:], e_t[:], e2[:], op=OP.max)
            nc.scalar.activation(expT[j][:], e_t[:], AF.Exp)

    # att_h natural [j, h] (lhsT for the ao matmul); copies alternate
    # ACT/DVE (GPSIMD cannot read PSUM)
    for ji, j in enumerate(JORD):
        ps = psumA2.tile([JW, H], F32, tag="ah", name="ah", bufs=3)
        jsl = slice(j * JW, (j + 1) * JW)
        for k in range(KA):
            nc.tensor.matmul(
                ps[:], attrT[k][:, jsl], att_w[k][:],
                start=(k == 0), stop=(k == KA - 1),
            )
        if ji % 2 == 0:
            nc.scalar.copy(att_h[j][:], ps[:])
        else:
            nc.vector.tensor_copy(att_h[j][:], ps[:])

    # fillers bridge the att_h-copy drain into wave A at full p-state
    for _ in range(9):
        nc.tensor.matmul(ps_w[:], junk_st[:], junk_mv[:],
                         start=True, stop=True)

    psumA2_ctx.__exit__(None, None, None)
    psumA1_ctx.__exit__(None, None, None)

    # ---- phase C: colsum + unnormalized att_outs^T, in two ih waves
    # with j innermost so expT[j] chunks are consumed as phase B
    # produces them ----
    with tc.tile_pool(name="psumB", bufs=1, space="PSUM") as psumB:
        cs_row = epool.tile([1, N], F32, tag="cs_row", name="cs_row")
        ps_cs = [
            psumB.tile([1, IW], F32, tag=f"cs{ih}", name=f"cs{ih}")
            for ih in range(2)
        ]
        ps_ao = [
            psumB.tile([P, IW], F32, tag=f"ao{m}", name=f"ao{m}")
            for m in range(HM)
        ]
        ps_w2 = psumB.tile([2, 512], F32, tag="warm2", name="warm2")
        # wave A (ih=0): colsum (both halves) + ao half 0, JORD order
        for ji, j in enumerate(JORD):
            for ih in range(2):
                isl = slice(ih * IW, (ih + 1) * IW)
                nc.tensor.matmul(
                    ps_cs[ih][:], ones_colb[:], expT[j][:, isl],
                    start=(ji == 0), stop=(ji == NJ - 1),
                )
            for m in range(HM):
                nc.tensor.matmul(
                    ps_ao[m][:], att_h[j][:, m * P:(m + 1) * P],
                    expT[j][:, 0:IW],
                    start=(ji == 0), stop=(ji == NJ - 1),
                )
        for ih in range(2):
            nc.vector.tensor_copy(
                cs_row[:, ih * IW:(ih + 1) * IW], ps_cs[ih][:]
            )
        recip_f = epool.tile([1, N], F32, tag="recip_f", name="recip_f")
        rc_scr = epool.tile([1, N], F32, tag="rc_scr", name="rc_scr")
        nc.vector.reciprocal_approx_accurate(
            out=recip_f[:], in_=cs_row[:], scratch=rc_scr[:]
        )
        nc.vector.tensor_copy(recip_bf[:], recip_f[:])
        for m in range(HM):
            if m % 2 == 0:
                nc.scalar.copy(aoT[m][:, 0:IW], ps_ao[m][:])
            else:
                nc.vector.tensor_copy(aoT[m][:, 0:IW], ps_ao[m][:])
        # wave B (ih=1); junk fillers cover the bank-release wait so the
        # PE p-state never resets
        for _ in range(N_FILL):
            nc.tensor.matmul(ps_w2[:], junk_st[:], junk_mv[:],
                             start=True, stop=True)
        ps_ao2 = [
            psumB.tile([P, IW], F32, tag=f"ao{m}", name=f"ao{m}b")
            for m in range(HM)
        ]
        for ji, j in enumerate(JORD):
            for m in range(HM):
                nc.tensor.matmul(
                    ps_ao2[m][:], att_h[j][:, m * P:(m + 1) * P],
                    expT[j][:, IW:N],
                    start=(ji == 0), stop=(ji == NJ - 1),
                )
        for m in range(HM):
            if m % 2 == 0:
                nc.scalar.copy(aoT[m][:, IW:N], ps_ao2[m][:])
            else:
                nc.vector.tensor_copy(aoT[m][:, IW:N], ps_ao2[m][:])
        # broadcast recip row to 128 partitions (bf16)
        for ih in range(2):
            isl = slice(ih * IW, (ih + 1) * IW)
            ps_rb = psumB.tile([P, IW], F32, tag=f"cs{ih}", name=f"rbp{ih}")
            nc.tensor.matmul(ps_rb[:], ones_rowb[:], recip_bf[:, isl])
            nc.vector.tensor_copy(rb_sb[ih][:], ps_rb[:])
        # fillers cover the psumB bank drain so img/sem2 dispatch at
        # full p-state
        for _ in range(8):
            nc.tensor.matmul(ps_w2[:], junk_st[:], junk_mv[:],
                             start=True, stop=True)

    # ---- phase A2: img_proj (|w|-scaled via img_w) + sem_b fold ----
    psumI_ctx = tc.tile_pool(name="psumI", bufs=1, space="PSUM")
    psumI = psumI_ctx.__enter__()
    if True:
        for m in range(HM):
            ps = psumI.tile([P, BS], F32, tag="img", name="img", bufs=1)
            msl = slice(m * P, (m + 1) * P)
            for k in range(KA):
                nc.tensor.matmul(
                    ps[:], img_w[k][:, msl], imgfTa[:, k * BS:(k + 1) * BS],
                    start=(k == 0), stop=(k == KA - 1),
                )
            nc.scalar.activation(
                imgb[m][:], ps[:], AF.Identity, bias=sem_bwa[:, m:m + 1]
            )
        ps = psumI.tile([BS, 1], F32, tag="fcbp", name="fcbp")
        nc.tensor.matmul(ps[:], ones_row[0:1, 0:BS], fcb[0:1, 0:1])
        nc.vector.tensor_copy(fcb_rep[:], ps[:])
        ps_w3 = psumI.tile([2, 512], F32, tag="warmI", name="warmI")

    # ---- phase D + E interleaved by m-chunk pair ----
    epool_ctx.__exit__(None, None, None)
    rpool_ctx = tc.tile_pool(name="relu", bufs=16)
    rpool = rpool_ctx.__enter__()
    zpool_ctx = tc.tile_pool(name="zfold", bufs=8)
    zpool = zpool_ctx.__enter__()

    pat = _producer_engines()
    pi = 0

    def producer(dst, m, b):
        nonlocal pi
        eng = pat[pi % len(pat)]
        pi += 1
        bias = imgb[m][:, b:b + 1]
        if eng == "A":
            nc.scalar.activation(dst[:], sem2T[m][:], AF.Relu, bias=bias)
        elif eng == "D":
            nc.vector.tensor_scalar(
                dst[:], sem2T[m][:], bias, 0.0, op0=OP.add, op1=OP.max
            )
        else:
            nc.gpsimd.tensor_scalar(
                dst[:], sem2T[m][:], bias, 0.0, op0=OP.add, op1=OP.max
            )

    psumC_ctx = tc.tile_pool(name="psumC", bufs=1, space="PSUM")
    psumC = psumC_ctx.__enter__()
    psumD_ctx = tc.tile_pool(name="psumD", bufs=1, space="PSUM")
    psumD = psumD_ctx.__enter__()
    out_ps = [
        psumD.tile([BS, IW], F32, tag=f"out{ih}", name=f"out{ih}")
        for ih in range(2)
    ]

    sem2_cnt = [0]

    def sem2_chunk(m):
        """sem2T'[m] = (sem_w'^T @ aoT) * recip  (fp16 out)."""
        msl = slice(m * P, (m + 1) * P)
        for ih in range(2):
            isl = slice(ih * IW, (ih + 1) * IW)
            ps = psumC.tile([P, IW], F32, tag="s2", name="s2", bufs=3)
            for k in range(KA):
                nc.tensor.matmul(
                    ps[:], sem_w[k][:, msl], aoT[k][:, isl],
                    start=(k == 0), stop=(k == KA - 1),
                )
            nc.vector.tensor_tensor(
                sem2T[m][:, isl], ps[:], rb_sb[ih][:], op=OP.mult
            )
            sem2_cnt[0] += 1

    # moving-operand count per ih: folded b -> 1, unfolded -> 2 per group
    n_mv = 2 * (FB + 2 * (BS - FB))
    mv_idx = [0]

    def e_matmul(stat_win, b, mv):
        """One reduce matmul pair into out_ps (accumulating)."""
        for ih in range(2):
            isl = slice(ih * IW, (ih + 1) * IW)
            nc.tensor.matmul(
                out_ps[ih][:], stat_win[:, 31 - b:63 - b], mv[:, isl],
                start=(mv_idx[0] == 0), stop=(mv_idx[0] == n_mv - 1),
            )
        mv_idx[0] += 1

    def phase_e_group(q):
        c0, c1 = 2 * q, 2 * q + 1
        for b in range(BS):
            if b in FOLDED:
                r0 = rpool.tile([P, N], F16, tag="r", name="r")
                r1 = rpool.tile([P, N], F16, tag="r", name="r")
                producer(r0, c0, b)
                producer(r1, c1, b)
                z = zpool.tile([P, N], F16, tag="z", name="z")
                nc.vector.tensor_tensor(z[:], r0[:], r1[:], op=OP.add)
                e_matmul(win_c[q], b, z)
            else:
                for c in (c0, c1):
                    r = rpool.tile([P, N], F16, tag="r", name="r")
                    producer(r, c, b)
                    e_matmul(win_s[c], b, r)

    for m in range(HM):
        sem2_chunk(m)
    for _ in range(6):
        nc.tensor.matmul(ps_w3[:], junk_st[:], junk_mv[:],
                         start=True, stop=True)
    phase_e_group(0)
    phase_e_group(1)

    nc.vector.tensor_scalar(
        out_sb[:, 0:IW], out_ps[0][:], fcb_rep[:, 0:1], None, op0=OP.add
    )
    nc.scalar.activation(
        out_sb[:, IW:N], out_ps[1][:], AF.Identity, bias=fcb_rep[:, 0:1],
    )
    nc.sync.dma_start(d_out[:, :], out_sb[:])

    psumD_ctx.__exit__(None, None, None)
    psumC_ctx.__exit__(None, None, None)
    psumI_ctx.__exit__(None, None, None)
    zpool_ctx.__exit__(None, None, None)
    rpool_ctx.__exit__(None, None, None)
    cpool_ctx.__exit__(None, None, None)


def _prepare_in_maps(image_feats, attributes, att_w, att_a, img_w, sem_w,
                     sem_b, fc_w, fc_b):
    f = np.float32
    bf = ml_dtypes.bfloat16
    attributes = np.asarray(attributes, f)
    att_w = np.asarray(att_w, f)
    att_a = np.asarray(att_a, f)
    image_feats = np.asarray(image_feats, f)
    sem_w = np.asarray(sem_w, f)
    img_w = np.asarray(img_w, f)
    sem_b = np.asarray(sem_b, f).reshape(H)
    fc_w = np.asarray(fc_w, f).reshape(H)
    fc_b = np.asarray(fc_b, f).reshape(1, 1)

    attrT = np.ascontiguousarray(attributes.T).astype(bf)       # [A, N]
    a1, a2 = att_a[:H, 0], att_a[H:, 0]
    w1 = (att_w @ a1).astype(f)                                 # [A]
    w2 = (att_w @ a2).astype(f)                                 # [A]
    # w1 broadcast chunks: w1b[k][a, p] = w1[k*128+a] for all p
    w1b = np.repeat(
        w1.reshape(KA, P, 1), P, axis=2
    ).transpose(1, 0, 2).reshape(P, KA * P).astype(bf)
    w1b = np.ascontiguousarray(w1b)
    w2p = np.ascontiguousarray(
        w2.reshape(KA, P).T
    ).astype(bf)                                                # [128, KA]

    # ---- sign/permutation machinery for the relation reduce ----
    w = fc_w.astype(np.float64).copy()
    sg = np.sign(w)
    if (sg > 0).sum() % 2 == 1:
        w[np.argmin(np.abs(w))] = 0.0
        sg = np.sign(w)
    pos = list(np.where(sg > 0)[0])
    neg = list(np.where(sg < 0)[0])
    wc = list(np.where(sg == 0)[0])  # 0 or 1 wildcards
    couples = []
    csigns = []
    for lst, s in ((pos, 1.0), (neg, -1.0)):
        while len(lst) >= 2:
            couples.append((lst.pop(), lst.pop()))
            csigns.append(s)
        if len(lst) == 1:
            couples.append((lst.pop(), wc.pop()))
            csigns.append(s)
    while len(couples) < 2 * P:  # only if many zero weights
        couples.append((wc.pop(), wc.pop()))
        csigns.append(0.0)
    assert len(couples) == 2 * P, len(couples)

    h_ord = np.zeros((HM, P), np.int64)
    s_chunk = np.zeros((HM, P), f)
    c_sign = np.zeros((2, P), f)
    for k, ((ha, hb), s) in enumerate(zip(couples, csigns)):
        q, p = k // P, k % P
        h_ord[2 * q][p] = ha
        h_ord[2 * q + 1][p] = hb
        s_chunk[2 * q][p] = sg[ha] if sg[ha] != 0 else 0.0
        s_chunk[2 * q + 1][p] = sg[hb] if sg[hb] != 0 else 0.0
        c_sign[q][p] = s
    perm = h_ord.reshape(H)
    aw = np.abs(w).astype(f)[perm]                              # |w| permuted

    # fold |w| into sem_w / img_w columns (permuted), sem_b
    sem_wp = (sem_w[:, perm] * aw[None, :]).astype(bf)
    img_wp = (img_w[:, perm] * aw[None, :]).astype(bf)
    sem_bw = (sem_b[perm] * aw).reshape(HM, P).T.astype(f)      # [128, HM]
    sem_bw = np.ascontiguousarray(sem_bw)

    # sign windows [128, 6*63]: col 31 of each window = signs
    swin = np.zeros((P, 6, 63), f)
    for c in range(4):
        swin[:, c, 31] = s_chunk[c]
    swin[:, 4, 31] = c_sign[0]
    swin[:, 5, 31] = c_sign[1]
    swin = np.ascontiguousarray(
        swin.reshape(P, 6 * 63).astype(np.float16)
    )

    def pack_k(wm):
        return np.ascontiguousarray(
            np.asarray(wm, bf).reshape(KA, P, H).transpose(1, 0, 2)
            .reshape(P, KA * H)
        )
    att_w_packed = pack_k(att_w.astype(bf))
    sem_w_packed = pack_k(sem_wp)
    img_w_packed = pack_k(img_wp)

    shared = {
        "attrT": attrT, "att_w": att_w_packed, "w1b": w1b, "w2": w2p,
        "img_w": img_w_packed, "sem_w": sem_w_packed, "sem_bw": sem_bw,
        "swin": swin, "fc_b": fc_b,
    }
    in_maps = []
    for c in range(NCORES):
        imgfT = np.ascontiguousarray(
            image_feats[c * BS:(c + 1) * BS, :].T
            .reshape(KA, P, BS).transpose(1, 0, 2).reshape(P, KA * BS)
        ).astype(bf)
        in_maps.append(dict(shared, imgfT=imgfT))
    return in_maps


def _make_runner(nc, in_maps):
    """Build the sharded PJRT callable once (mirrors
    bass2jax.run_bass_via_pjrt's multi-core path) so repeated kernel()
    calls reuse the compiled NEFF executable."""
    import jax
    from jax.sharding import Mesh, PartitionSpec

    try:
        from jax.experimental.shard_map import shard_map
    except ImportError:
        shard_map = jax.shard_map
    from concourse import bass2jax

    bass2jax.install_neuronx_cc_hook()
    n_cores = len(in_maps)
    partition_name = (
        nc.partition_id_tensor.name if nc.partition_id_tensor else None
    )
    in_names, out_names, out_avals = [], [], []
    for alloc in nc.m.functions[0].allocations:
        if not isinstance(alloc, mybir.MemoryLocationSet):
            continue
        name = alloc.memorylocations[0].name
        if alloc.kind == "ExternalInput":
            if name != partition_name:
                in_names.append(name)
        elif alloc.kind == "ExternalOutput":
            out_names.append(name)
            out_avals.append(
                jax.core.ShapedArray(
                    tuple(alloc.tensor_shape), mybir.dt.np(alloc.dtype)
                )
            )
    all_in_names = list(in_names) + list(out_names)
    if partition_name is not None:
        all_in_names.append(partition_name)
    n_params, n_outs = len(in_names), len(out_avals)

    def _body(*args):
        operands = list(args)
        if partition_name is not None:
            operands.append(bass2jax.partition_id_tensor())
        return tuple(bass2jax._bass_exec_p.bind(
            *operands,
            out_avals=tuple(out_avals),
            in_names=tuple(all_in_names),
            out_names=tuple(out_names),
            lowering_input_output_aliases=(),
            sim_require_finite=True,
            sim_require_nnan=True,
            nc=nc,
        ))

    donate = tuple(range(n_params, n_params + n_outs))
    devices = jax.devices()[:n_cores]
    mesh = Mesh(np.asarray(devices), ("core",))
    sharded = jax.jit(
        shard_map(
            _body, mesh=mesh,
            in_specs=(PartitionSpec("core"),) * (n_params + n_outs),
            out_specs=(PartitionSpec("core"),) * n_outs,
            check_rep=False,
        ),
        donate_argnums=donate, keep_unused=True,
    )

    import zlib

    def call(maps):
        concat_in = [
            np.concatenate([np.asarray(maps[c][n]) for c in range(n_cores)], 0)
            for n in in_names
        ]
        key = tuple(zlib.adler32(x.tobytes()) for x in concat_in)
        dev = _CACHE.get("dev_inputs")
        if dev is None or dev[0] != key:
            dev = (key, [jax.device_put(x) for x in concat_in])
            _CACHE["dev_inputs"] = dev
        zeros = [
            np.zeros((n_cores * av.shape[0], *av.shape[1:]), av.dtype)
            for av in out_avals
        ]
        outs = sharded(*dev[1], *zeros)
        jax.block_until_ready(outs)
        oi = out_names.index("out")
        full = np.asarray(outs[oi]).reshape(n_cores, *out_avals[oi].shape)
        return np.concatenate(list(full), axis=0).astype(np.float32)

    return call


def run(inputs, **spmd_kwargs):
    """Returns (full output [B, N], BassKernelResults) via the generic
    run_bass_kernel_spmd path (used by test tooling)."""
    nc = _build_program()
    in_maps = _prepare_in_maps(**inputs)
    res = run_bass_kernel_spmd(nc, in_maps, list(range(NCORES)), **spmd_kwargs)
    out = np.concatenate(
        [res.results[c]["out"] for c in range(NCORES)], axis=0
    ).astype(np.float32)
    return out, res


def kernel(**inputs):
    nc = _build_program()
    in_maps = _prepare_in_maps(**inputs)
    if "runner" not in _CACHE:
        _CACHE["runner"] = _make_runner(nc, in_maps)
    return _CACHE["runner"](in_maps)
